# revision 5
# baseline (speedup 1.0000x reference)
"""Trainium2 Bass kernel for the merged multi-adapter LoRA layer.

Math (all fp32):
    t[n,b,j,d]  = sum_m x[b,j,m] * lora_A[n,d,m]
    out[n,b,j,k] = sum_d t[n,b,j,d] * lora_B[n,k,d]

Shapes: x (4,2048,4096), lora_A (4,16,4096), lora_B (4,4096,16)
        out (4,4,2048,4096)

Sharding: data-parallel over flattened tokens (b*j = 8192 -> 1024/core on
8 cores); the tiny LoRA params are replicated.

This problem is HBM-bound on the output write, so all device I/O is fp16
(well inside the 2e-2 gate: fp16 quantization of out adds ~3e-4 rel err):
  - x is cast to fp16 AND pre-transposed on the host into per-token-tile
    packed form xs[m%128][32*tok0 + mt*w + tok] so each token tile is ONE
    contiguous DMA and mm1 needs no on-chip transpose.
  - out is written fp16 (32 MiB/core instead of 64) and upcast on host.

Per-core dataflow (Tile framework):
  - mm1: t^T[c, tok] = sum_mt A_pack[m, c]^T @ xT[m, tok], c = 32*n + d
    packs all 4 adapters into one 128-wide output (cols 16..31 of each
    32-block are zero so mm2 tile_positions land on rows 0/32/64/96;
    those t rows are never read by mm2).
  - mm2: out[tok, k] = t^T[32n+d, tok]^T @ B_pack[n, d, k]; the D=16
    contraction uses PE row-band tile_position packing per adapter.
  - PSUM results are converted fp32->fp16 into wide [128, 4096] staging
    tiles with 1024-wide casts (vector/scalar alternating) and DMA'd out
    as 1 MiB contiguous stores.
  - Token tiles ramp [128,128,256,256,256] and x loads are issued one
    tile ahead, so the first store hits the queues ~20us in instead of
    waiting for all input descriptors to drain.
"""

import numpy as np

import concourse.bacc as bacc
import concourse.mybir as mybir
import concourse.tile as tile
from concourse import bass_utils
from concourse.bass import ds, ts

F32 = mybir.dt.float32
F16 = mybir.dt.float16

N_CORES = 8
B, J, M = 4, 2048, 4096
N, D, K = 4, 16, 4096
TOK = B * J                      # 8192 flattened tokens
TOK_PER_CORE = TOK // N_CORES    # 1024
MT = 128                         # m (contraction) tile
N_MT = M // MT                   # 32
KT = 512                         # matmul k tile (one PSUM bank of fp32)
KC = 1024                        # cast width (2 PSUM banks)
ADP = 32                         # partition stride per adapter in packed dim
TS = [128, 128, 256, 256, 256]   # ramped token tiles
assert sum(TS) == TOK_PER_CORE


def build_program():
    nc = bacc.Bacc("TRN2")

    xs = nc.dram_tensor(
        "xs", [128, N_MT * TOK_PER_CORE], F16, kind="ExternalInput"
    ).ap()
    a_p = nc.dram_tensor("a_p", [128, N_MT, MT], F16, kind="ExternalInput").ap()
    b_p = nc.dram_tensor("b_p", [N, D, K], F16, kind="ExternalInput").ap()
    o = nc.dram_tensor("o", [N, TOK_PER_CORE, K], F16, kind="ExternalOutput").ap()

    with tile.TileContext(nc) as tc:
        with (
            tc.tile_pool(name="apool", bufs=1) as apool,
            tc.tile_pool(name="bpool", bufs=1) as bpool,
            tc.tile_pool(name="xpool", bufs=1) as xpool,
            tc.tile_pool(name="tpool", bufs=2) as tpool,
            tc.tile_pool(name="opool", bufs=8) as opool,
            tc.tile_pool(name="tps", bufs=2, space="PSUM") as tps_pool,
            tc.tile_pool(name="ops", bufs=3, space="PSUM") as ops_pool,
        ):
            # issue order: a, x0, b, x1, then x_{t+1} right before mm1 of
            # tile t so at most ~1 tile of loads queues ahead of stores
            a_sb = apool.tile([128, N_MT, MT], F16, tag="a")
            nc.scalar.dma_start(a_sb[:], a_p[:])

            def load_x(t):
                w = TS[t]
                tok0 = sum(TS[:t])
                xt = xpool.tile(
                    [128, N_MT * w], F16, tag=f"x{w}", name=f"xt{t}",
                    bufs=sum(1 for v in TS if v == w),
                )
                nc.scalar.dma_start(
                    xt[:], xs[:, ds(N_MT * tok0, N_MT * w)]
                )
                return xt

            xts = {0: load_x(0)}

            b_sb = bpool.tile([128, K], F16, tag="b")
            for n in range(N):
                nc.scalar.dma_start(b_sb[ds(ADP * n, D), :], b_p[n])

            xts[1] = load_x(1)

            for t, w in enumerate(TS):
                if t + 2 < len(TS):
                    xts[t + 2] = load_x(t + 2)
                xt = xts.pop(t)

                t_ps = tps_pool.tile([128, w], F32, tag="tps", name="tps")
                for mt in range(N_MT):
                    nc.tensor.matmul(
                        t_ps[:],
                        lhsT=a_sb[:, mt, :],
                        rhs=xt[:, ds(mt * w, w)],
                        start=(mt == 0),
                        stop=(mt == N_MT - 1),
                    )
                t_sb = tpool.tile([128, w], F16, tag="t", name="tsb")
                nc.vector.tensor_copy(t_sb[:], t_ps[:])

                for s in range(w // 128):
                    tok_abs = sum(TS[:t]) + s * 128
                    i = 0
                    for n in range(N):
                        osb = opool.tile([128, K], F16, tag="o", name="osb")
                        for kc in range(K // KC):
                            o_ps = ops_pool.tile(
                                [128, KC], F32, tag="ops", name="ops"
                            )
                            for h in range(KC // KT):
                                nc.tensor.matmul(
                                    o_ps[:, ts(h, KT)],
                                    lhsT=t_sb[ds(ADP * n, D), ts(s, 128)],
                                    rhs=b_sb[
                                        ds(ADP * n, D), ds(kc * KC + h * KT, KT)
                                    ],
                                    start=True,
                                    stop=True,
                                    tile_position=(ADP * n, 0),
                                )
                            if i % 2 == 0:
                                nc.vector.tensor_copy(osb[:, ts(kc, KC)], o_ps[:])
                            else:
                                nc.scalar.copy(osb[:, ts(kc, KC)], o_ps[:])
                            i += 1
                        nc.sync.dma_start(o[n, ds(tok_abs, 128), :], osb[:])

    nc.compile()
    return nc


_NC_CACHE = []


def _get_nc():
    if not _NC_CACHE:
        _NC_CACHE.append(build_program())
    return _NC_CACHE[0]


def prepare_inputs(x, lora_A, lora_B):
    x = np.asarray(x, dtype=np.float32).astype(np.float16)
    lora_A = np.asarray(lora_A, dtype=np.float32)
    lora_B = np.asarray(lora_B, dtype=np.float32)

    # xs[core, p, 32*tok0 + mt*w + c] = x[core*1024 + tok0 + c, mt*128 + p]
    xf = x.reshape(TOK, M)
    xs_parts = []
    for t, w in enumerate(TS):
        tok0 = sum(TS[:t])
        blk = xf.reshape(N_CORES, TOK_PER_CORE, N_MT, MT)[:, tok0 : tok0 + w]
        xs_parts.append(blk.transpose(0, 3, 2, 1).reshape(N_CORES, 128, N_MT * w))
    xs = np.ascontiguousarray(np.concatenate(xs_parts, axis=2))

    # a_pack[p, mt, c] with a_t[m, 32n+d] = lora_A[n, d, m]
    a_t = np.zeros((M, 128), dtype=np.float32)
    for n in range(N):
        a_t[:, ADP * n : ADP * n + D] = lora_A[n].T
    a_pack = np.ascontiguousarray(
        a_t.reshape(N_MT, MT, 128).transpose(1, 0, 2)
    ).astype(np.float16)

    # b_pack[n, d, k] = lora_B[n, k, d]
    b_pack = np.ascontiguousarray(lora_B.transpose(0, 2, 1)).astype(np.float16)

    in_maps = [
        {"xs": xs[c], "a_p": a_pack, "b_p": b_pack}
        for c in range(N_CORES)
    ]
    return in_maps


def run(x, lora_A, lora_B, trace=False, **spmd_kwargs):
    nc = _get_nc()
    in_maps = prepare_inputs(x, lora_A, lora_B)
    res = bass_utils.run_bass_kernel_spmd(
        nc, in_maps, list(range(N_CORES)), trace=trace, **spmd_kwargs
    )
    o_full = np.concatenate(
        [res.results[c]["o"].astype(np.float32) for c in range(N_CORES)], axis=1
    )
    return o_full.reshape(N, B, J, K), res


def kernel(x, lora_A, lora_B):
    out, _ = run(x, lora_A, lora_B)
    return out


# revision 6
# speedup vs baseline: 1.0667x; 1.0667x over previous
"""Trainium2 Bass kernel for the merged multi-adapter LoRA layer.

Math (all fp32):
    t[n,b,j,d]  = sum_m x[b,j,m] * lora_A[n,d,m]
    out[n,b,j,k] = sum_d t[n,b,j,d] * lora_B[n,k,d]

Shapes: x (4,2048,4096), lora_A (4,16,4096), lora_B (4,4096,16)
        out (4,4,2048,4096)

Sharding: data-parallel over flattened tokens (b*j = 8192 -> 1024/core on
8 cores); the tiny LoRA params are replicated.

This problem is HBM-bound on the output write, so all device I/O is fp16
(well inside the 2e-2 gate: fp16 quantization of out adds ~3e-4 rel err):
  - x is cast to fp16 AND pre-transposed on the host into per-token-tile
    packed form xs[m%128][32*tok0 + mt*w + tok] so each token tile loads
    with contiguous DMA lines and mm1 needs no on-chip transpose.
  - out is written fp16 (32 MiB/core instead of 64) and upcast on host.
  - lora_B loads only its 16 non-zero rows per adapter band.

Per-core dataflow (Tile framework):
  - mm1: t^T[c, tok] = sum_mt A_pack[m, c]^T @ xT[m, tok], c = 32*n + d
    packs all 4 adapters into one 128-wide output (cols 16..31 of each
    32-block are zero so mm2 tile_positions land on rows 0/32/64/96;
    those t rows are never read by mm2).
  - mm2: out[tok, k] = t^T[32n+d, tok]^T @ B_pack[n, d, k]; the D=16
    contraction uses PE row-band tile_position packing, adapter-rotated
    (n innermost) so each matmul's LDWEIGHTS overlaps the previous
    matmul on a different 32-row band.
  - PSUM evacuation: fp32->fp16 512-wide casts alternating Vector/Scalar
    into [128, 4096] staging tiles; stores split in 2 KiB halves so the
    write stream starts as soon as half a tile is cast.
  - Token tiles ramp [128, 256, 256, 384]; x loads are issued one tile
    ahead so stores never queue behind the whole input stream.
"""

import numpy as np

import concourse.bacc as bacc
import concourse.mybir as mybir
import concourse.tile as tile
from concourse import bass_utils
from concourse.bass import ds, ts

F32 = mybir.dt.float32
F16 = mybir.dt.float16

N_CORES = 8
B, J, M = 4, 2048, 4096
N, D, K = 4, 16, 4096
TOK = B * J                      # 8192 flattened tokens
TOK_PER_CORE = TOK // N_CORES    # 1024
MT = 128                         # m (contraction) tile
N_MT = M // MT                   # 32
KT = 512                         # matmul k tile (one PSUM bank of fp32)
ADP = 32                         # partition stride per adapter in packed dim
TS = [128, 256, 256, 384]        # ramped token tiles
OH = 2048                        # store half-width
assert sum(TS) == TOK_PER_CORE


def build_program():
    nc = bacc.Bacc("TRN2")

    xs = nc.dram_tensor(
        "xs", [128, N_MT * TOK_PER_CORE], F16, kind="ExternalInput"
    ).ap()
    a_p = nc.dram_tensor("a_p", [128, N_MT, MT], F16, kind="ExternalInput").ap()
    b_p = nc.dram_tensor("b_p", [N, D, K], F16, kind="ExternalInput").ap()
    o = nc.dram_tensor("o", [N, TOK_PER_CORE, K], F16, kind="ExternalOutput").ap()

    with tile.TileContext(nc) as tc:
        with (
            tc.tile_pool(name="apool", bufs=1) as apool,
            tc.tile_pool(name="bpool", bufs=1) as bpool,
            tc.tile_pool(name="xpool", bufs=1) as xpool,
            tc.tile_pool(name="tpool", bufs=2) as tpool,
            tc.tile_pool(name="opool", bufs=8) as opool,
            tc.tile_pool(name="tps", bufs=2, space="PSUM") as tps_pool,
            tc.tile_pool(name="ops", bufs=6, space="PSUM") as ops_pool,
        ):
            a_sb = apool.tile([128, N_MT, MT], F16, tag="a")
            nc.scalar.dma_start(a_sb[:], a_p[:])

            def load_x(t):
                w = TS[t]
                tok0 = sum(TS[:t])
                xt = xpool.tile(
                    [128, N_MT * w], F16, tag=f"x{w}", name=f"xt{t}",
                    bufs=sum(1 for v in TS if v == w),
                )
                # two chunks (16 m-tiles each) for finer mm1 dependencies
                half = N_MT * w // 2
                nc.scalar.dma_start(
                    xt[:, ds(0, half)], xs[:, ds(N_MT * tok0, half)]
                )
                nc.scalar.dma_start(
                    xt[:, ds(half, half)], xs[:, ds(N_MT * tok0 + half, half)]
                )
                return xt

            xts = {0: load_x(0)}

            b_sb = bpool.tile([128, K], F16, tag="b")
            for n in range(N):
                nc.scalar.dma_start(b_sb[ds(ADP * n, D), :], b_p[n])

            xts[1] = load_x(1)

            for t, w in enumerate(TS):
                if t + 2 < len(TS):
                    xts[t + 2] = load_x(t + 2)
                xt = xts.pop(t)

                t_ps = tps_pool.tile([128, w], F32, tag="tps", name="tps")
                for mt in range(N_MT):
                    nc.tensor.matmul(
                        t_ps[:],
                        lhsT=a_sb[:, mt, :],
                        rhs=xt[:, ds(mt * w, w)],
                        start=(mt == 0),
                        stop=(mt == N_MT - 1),
                    )
                t_sb = tpool.tile([128, w], F16, tag="t", name="tsb")
                nc.vector.tensor_copy(t_sb[:], t_ps[:])

                for s in range(w // 128):
                    tok_abs = sum(TS[:t]) + s * 128
                    osb = [
                        opool.tile([128, K], F16, tag="o", name="osb")
                        for _ in range(N)
                    ]
                    i = 0
                    for kt in range(K // KT):
                        for n in range(N):
                            o_ps = ops_pool.tile(
                                [128, KT], F32, tag="ops", name="ops"
                            )
                            nc.tensor.matmul(
                                o_ps[:],
                                lhsT=t_sb[ds(ADP * n, D), ts(s, 128)],
                                rhs=b_sb[ds(ADP * n, D), ts(kt, KT)],
                                start=True,
                                stop=True,
                                tile_position=(ADP * n, 0),
                            )
                            if i % 2 == 0:
                                nc.vector.tensor_copy(osb[n][:, ts(kt, KT)], o_ps[:])
                            else:
                                nc.scalar.copy(osb[n][:, ts(kt, KT)], o_ps[:])
                            i += 1
                    for n in range(N):
                        for h in range(K // OH):
                            nc.sync.dma_start(
                                o[n, ds(tok_abs, 128), ds(h * OH, OH)],
                                osb[n][:, ds(h * OH, OH)],
                            )

    nc.compile()
    return nc


_NC_CACHE = []


def _get_nc():
    if not _NC_CACHE:
        _NC_CACHE.append(build_program())
    return _NC_CACHE[0]


def prepare_inputs(x, lora_A, lora_B):
    x = np.asarray(x, dtype=np.float32).astype(np.float16)
    lora_A = np.asarray(lora_A, dtype=np.float32)
    lora_B = np.asarray(lora_B, dtype=np.float32)

    # xs[core, p, 32*tok0 + mt*w + c] = x[core*1024 + tok0 + c, mt*128 + p]
    xf = x.reshape(TOK, M)
    xs_parts = []
    for t, w in enumerate(TS):
        tok0 = sum(TS[:t])
        blk = xf.reshape(N_CORES, TOK_PER_CORE, N_MT, MT)[:, tok0 : tok0 + w]
        xs_parts.append(blk.transpose(0, 3, 2, 1).reshape(N_CORES, 128, N_MT * w))
    xs = np.ascontiguousarray(np.concatenate(xs_parts, axis=2))

    # a_pack[p, mt, c] with a_t[m, 32n+d] = lora_A[n, d, m]
    a_t = np.zeros((M, 128), dtype=np.float32)
    for n in range(N):
        a_t[:, ADP * n : ADP * n + D] = lora_A[n].T
    a_pack = np.ascontiguousarray(
        a_t.reshape(N_MT, MT, 128).transpose(1, 0, 2)
    ).astype(np.float16)

    # b_pack[n, d, k] = lora_B[n, k, d]
    b_pack = np.ascontiguousarray(lora_B.transpose(0, 2, 1)).astype(np.float16)

    in_maps = [
        {"xs": xs[c], "a_p": a_pack, "b_p": b_pack}
        for c in range(N_CORES)
    ]
    return in_maps


def run(x, lora_A, lora_B, trace=False, **spmd_kwargs):
    nc = _get_nc()
    in_maps = prepare_inputs(x, lora_A, lora_B)
    res = bass_utils.run_bass_kernel_spmd(
        nc, in_maps, list(range(N_CORES)), trace=trace, **spmd_kwargs
    )
    o_full = np.concatenate(
        [res.results[c]["o"].astype(np.float32) for c in range(N_CORES)], axis=1
    )
    return o_full.reshape(N, B, J, K), res


def kernel(x, lora_A, lora_B):
    out, _ = run(x, lora_A, lora_B)
    return out


# revision 7
# speedup vs baseline: 1.2264x; 1.1497x over previous
"""Trainium2 Bass kernel for the merged multi-adapter LoRA layer.

Math (all fp32):
    t[n,b,j,d]  = sum_m x[b,j,m] * lora_A[n,d,m]
    out[n,b,j,k] = sum_d t[n,b,j,d] * lora_B[n,k,d]

Shapes: x (4,2048,4096), lora_A (4,16,4096), lora_B (4,4096,16)
        out (4,4,2048,4096)

Sharding: data-parallel over flattened tokens (b*j = 8192 -> 1024/core on
8 cores); the tiny LoRA params are replicated.

This problem is HBM-bound on the output write, so all device I/O is fp16
(well inside the 2e-2 gate: fp16 quantization of out adds ~3e-4 rel err):
  - x is cast to fp16 AND pre-transposed on the host into per-token-tile
    packed form xs[m%128][32*tok0 + mt*w + tok] so each token tile loads
    with contiguous DMA lines and mm1 needs no on-chip transpose.
  - out is written fp16 (32 MiB/core instead of 64) and upcast on host.
  - lora_B loads only its 16 non-zero rows per adapter band.

Per-core dataflow (Tile framework):
  - mm1: t^T[c, tok] = sum_mt A_pack[m, c]^T @ xT[m, tok], c = 32*n + d
    packs all 4 adapters into one 128-wide output (cols 16..31 of each
    32-block are zero so mm2 tile_positions land on rows 0/32/64/96;
    those t rows are never read by mm2).
  - mm2: out[tok, k] = t^T[32n+d, tok]^T @ B_pack[n, d, k]; the D=16
    contraction uses PE row-band tile_position packing, adapter-rotated
    (n innermost) so each matmul's LDWEIGHTS overlaps the previous
    matmul on a different 32-row band.
  - PSUM evacuation: fp32->fp16 512-wide casts alternating Vector/Scalar
    into [128, 4096] staging tiles; stores split in 2 KiB halves so the
    write stream starts as soon as half a tile is cast.
  - Token tiles ramp [128, 256, 256, 384]; x loads are issued one tile
    ahead so stores never queue behind the whole input stream.
"""

import numpy as np

import concourse.bacc as bacc
import concourse.mybir as mybir
import concourse.tile as tile
from concourse import bass_utils
from concourse.bass import ds, ts

F32 = mybir.dt.float32
F16 = mybir.dt.float16

N_CORES = 8
B, J, M = 4, 2048, 4096
N, D, K = 4, 16, 4096
TOK = B * J                      # 8192 flattened tokens
TOK_PER_CORE = TOK // N_CORES    # 1024
MT = 128                         # m (contraction) tile
N_MT = M // MT                   # 32
KT = 512                         # matmul k tile (one PSUM bank of fp32)
ADP = 32                         # partition stride per adapter in packed dim
TS = [128, 256, 256, 384]        # ramped token tiles
OH = 2048                        # store half-width
assert sum(TS) == TOK_PER_CORE


def build_program():
    nc = bacc.Bacc("TRN2")

    xs = nc.dram_tensor(
        "xs", [128, N_MT * TOK_PER_CORE], F16, kind="ExternalInput"
    ).ap()
    a_p = nc.dram_tensor("a_p", [128, N_MT, MT], F16, kind="ExternalInput").ap()
    b_p = nc.dram_tensor("b_p", [N, D, K], F16, kind="ExternalInput").ap()
    o = nc.dram_tensor("o", [N, TOK_PER_CORE, K], F16, kind="ExternalOutput").ap()

    with tile.TileContext(nc) as tc:
        with (
            tc.tile_pool(name="apool", bufs=1) as apool,
            tc.tile_pool(name="bpool", bufs=1) as bpool,
            tc.tile_pool(name="xpool", bufs=1) as xpool,
            tc.tile_pool(name="tpool", bufs=2) as tpool,
            tc.tile_pool(name="opool", bufs=8) as opool,
            tc.tile_pool(name="tps", bufs=2, space="PSUM") as tps_pool,
            tc.tile_pool(name="ops", bufs=6, space="PSUM") as ops_pool,
        ):
            a_sb = apool.tile([128, N_MT, MT], F16, tag="a")
            nc.scalar.dma_start(a_sb[:], a_p[:])

            def load_x(t):
                w = TS[t]
                tok0 = sum(TS[:t])
                xt = xpool.tile(
                    [128, N_MT * w], F16, tag=f"x{w}", name=f"xt{t}",
                    bufs=sum(1 for v in TS if v == w),
                )
                # two chunks (16 m-tiles each) for finer mm1 dependencies
                half = N_MT * w // 2
                nc.scalar.dma_start(
                    xt[:, ds(0, half)], xs[:, ds(N_MT * tok0, half)]
                )
                nc.scalar.dma_start(
                    xt[:, ds(half, half)], xs[:, ds(N_MT * tok0 + half, half)]
                )
                return xt

            xts = {0: load_x(0)}

            b_sb = bpool.tile([128, K], F16, tag="b")
            for n in range(N):
                nc.scalar.dma_start(b_sb[ds(ADP * n, D), :], b_p[n])

            # issue ALL loads upfront in consumption order: the DMA queues
            # serve descriptors in issue order, so compute never waits on a
            # late-issued load, and the queues hand off seamlessly from the
            # load stream to the (backlogged) store stream with no idle gap
            for t in range(1, len(TS)):
                xts[t] = load_x(t)

            for t, w in enumerate(TS):
                xt = xts.pop(t)

                t_ps = tps_pool.tile([128, w], F32, tag="tps", name="tps")
                for mt in range(N_MT):
                    nc.tensor.matmul(
                        t_ps[:],
                        lhsT=a_sb[:, mt, :],
                        rhs=xt[:, ds(mt * w, w)],
                        start=(mt == 0),
                        stop=(mt == N_MT - 1),
                    )
                t_sb = tpool.tile([128, w], F16, tag="t", name="tsb")
                nc.vector.tensor_copy(t_sb[:], t_ps[:])

                for s in range(w // 128):
                    tok_abs = sum(TS[:t]) + s * 128
                    osb = [
                        opool.tile([128, K], F16, tag="o", name="osb")
                        for _ in range(N)
                    ]
                    i = 0
                    for kt in range(K // KT):
                        for n in range(N):
                            o_ps = ops_pool.tile(
                                [128, KT], F32, tag="ops", name="ops"
                            )
                            nc.tensor.matmul(
                                o_ps[:],
                                lhsT=t_sb[ds(ADP * n, D), ts(s, 128)],
                                rhs=b_sb[ds(ADP * n, D), ts(kt, KT)],
                                start=True,
                                stop=True,
                                tile_position=(ADP * n, 0),
                            )
                            if i % 2 == 0:
                                nc.vector.tensor_copy(osb[n][:, ts(kt, KT)], o_ps[:])
                            else:
                                nc.scalar.copy(osb[n][:, ts(kt, KT)], o_ps[:])
                            i += 1
                    for n in range(N):
                        for h in range(K // OH):
                            nc.sync.dma_start(
                                o[n, ds(tok_abs, 128), ds(h * OH, OH)],
                                osb[n][:, ds(h * OH, OH)],
                            )

    nc.compile()
    return nc


_NC_CACHE = []


def _get_nc():
    if not _NC_CACHE:
        _NC_CACHE.append(build_program())
    return _NC_CACHE[0]


def prepare_inputs(x, lora_A, lora_B):
    x = np.asarray(x, dtype=np.float32).astype(np.float16)
    lora_A = np.asarray(lora_A, dtype=np.float32)
    lora_B = np.asarray(lora_B, dtype=np.float32)

    # xs[core, p, 32*tok0 + mt*w + c] = x[core*1024 + tok0 + c, mt*128 + p]
    xf = x.reshape(TOK, M)
    xs_parts = []
    for t, w in enumerate(TS):
        tok0 = sum(TS[:t])
        blk = xf.reshape(N_CORES, TOK_PER_CORE, N_MT, MT)[:, tok0 : tok0 + w]
        xs_parts.append(blk.transpose(0, 3, 2, 1).reshape(N_CORES, 128, N_MT * w))
    xs = np.ascontiguousarray(np.concatenate(xs_parts, axis=2))

    # a_pack[p, mt, c] with a_t[m, 32n+d] = lora_A[n, d, m]
    a_t = np.zeros((M, 128), dtype=np.float32)
    for n in range(N):
        a_t[:, ADP * n : ADP * n + D] = lora_A[n].T
    a_pack = np.ascontiguousarray(
        a_t.reshape(N_MT, MT, 128).transpose(1, 0, 2)
    ).astype(np.float16)

    # b_pack[n, d, k] = lora_B[n, k, d]
    b_pack = np.ascontiguousarray(lora_B.transpose(0, 2, 1)).astype(np.float16)

    in_maps = [
        {"xs": xs[c], "a_p": a_pack, "b_p": b_pack}
        for c in range(N_CORES)
    ]
    return in_maps


def run(x, lora_A, lora_B, trace=False, **spmd_kwargs):
    nc = _get_nc()
    in_maps = prepare_inputs(x, lora_A, lora_B)
    res = bass_utils.run_bass_kernel_spmd(
        nc, in_maps, list(range(N_CORES)), trace=trace, **spmd_kwargs
    )
    o_full = np.concatenate(
        [res.results[c]["o"].astype(np.float32) for c in range(N_CORES)], axis=1
    )
    return o_full.reshape(N, B, J, K), res


def kernel(x, lora_A, lora_B):
    out, _ = run(x, lora_A, lora_B)
    return out
